# revision 4
# baseline (speedup 1.0000x reference)
"""Correlation layer (FlowNet-style) on 8 Trainium2 NeuronCores.

Strategy (data-parallel over batch, one batch element per core):
  out[d,h,w] = sum_c x1[c,h,w] * x2p[c, h+di+4, w+dj+4] / sqrt(C),
  di,dj in [-4,4], 80 displacements (81 minus center).

Per core, banded-Gram matmuls with displacement packing:
  - lhsT = x1 rows [24g-8+rho'' : +32) x 4 w-cols  -> M = 32*4 = 128
    (m = rho''*4 + ww, rho'' = rr - di + 4)
  - rhs  = x2p rows [24g : 24g+24) x 12 padded-w cols -> N = 24*12 = 288
    (n = rr*12 + u, u = ww + dj + 4)
  - psum[m, n] is useful iff rho'' = rr - di + 4 and u - ww in [0, 9).
    For fixed rr all useful elements live in partitions [4rr, 4rr+36),
    so a pure-partition-step DMA can ship a 75%-dense window per rr.

Pipeline: DMA x2p slab (24 rows) -> 288-col f32r/bf16/f32 matmuls ->
DVE/ACT copy PSUM->SBUF staging (relayout to (rr, wb, u)) -> per-rr
window DMA (576B contiguous runs) -> DRAM; host decodes windows into
the [80, H, W] layout with pure slicing.
"""

import math
import numpy as np
from contextlib import ExitStack

B, C, H, W = 8, 128, 128, 192
MD = 4
NDISP = 81

R = 24          # x2p rows per group
NG = 6          # row groups (covers 144 padded rows)
WW = 4          # output w-cols per block
NWB = W // WW   # 48 blocks
CHUNK = 12      # blocks per staging chunk
NCH = NWB // CHUNK  # 4
HP = 152        # x1pad rows: 8 zero + 128 + 16 zero
X1B = NWB * 32 * WW        # per-group x1 block slab: 6144 elems/partition
X2R, X2C = NG * R, W + 8   # 144 x 200
UB = WW + 8     # 12 rhs cols per block
NMM = R * UB    # 288 matmul free size
ROWSZ = R * CHUNK * UB     # staging free size 3456
WIN = 36                   # band window partitions per rr
OUTSZ = NG * NCH * R * WIN * (CHUNK * UB)  # per-core band elements

MM_DTYPE = "float32"      # "float32" | "float32r" | "bfloat16"
EVAC_PATTERN = "vvs"       # per-copy engine cycle: v=vector, s=scalar

_CACHE = {}


def _build(mm_dtype, evac_pattern):
    import concourse.bass as bass
    import concourse.tile as tile
    from concourse import bacc, mybir

    in_dt = mybir.dt.bfloat16 if mm_dtype == "bfloat16" else mybir.dt.float32
    f32 = mybir.dt.float32

    nc = bacc.Bacc("TRN2", target_bir_lowering=False, debug=False, num_devices=8)
    x1d = nc.dram_tensor("x1p", [128, NG * X1B], in_dt, kind="ExternalInput")
    x2d = nc.dram_tensor("x2p", [128, X2R * X2C], in_dt, kind="ExternalInput")
    outd = nc.dram_tensor("band", [OUTSZ], f32, kind="ExternalOutput")

    with tile.TileContext(nc) as tc, ExitStack() as ctx:
        x1pool = ctx.enter_context(tc.tile_pool(name="x1", bufs=2))
        x2pool = ctx.enter_context(tc.tile_pool(name="x2", bufs=2))
        pspool = ctx.enter_context(tc.tile_pool(name="ps", bufs=3, space="PSUM"))
        stpool = ctx.enter_context(tc.tile_pool(name="st", bufs=2))

        ev = 0
        for g in range(NG):
            x1t = x1pool.tile([128, X1B], in_dt, tag="x1")
            nc.sync.dma_start(x1t[:], x1d.ap()[:, g * X1B:(g + 1) * X1B])
            x2t = x2pool.tile([128, R * X2C], in_dt, tag="x2")
            nc.sync.dma_start(x2t[:], x2d.ap()[:, g * R * X2C:(g + 1) * R * X2C])
            x2v = x2t[:].rearrange("p (r u) -> p r u", r=R)
            for chn in range(NCH):
                stt = stpool.tile([128, ROWSZ], f32, tag="st")
                stv = stt[:].rearrange("p (r b u) -> p r b u", r=R, b=CHUNK)
                for wp in range(CHUNK // 2):
                    pst = pspool.tile([128, 2, 512], f32, tag="ps")
                    for k in range(2):
                        wb = chn * CHUNK + wp * 2 + k
                        lhsT = x1t[:, wb * 128:(wb + 1) * 128]
                        rhs = x2v[:, :, wb * WW:wb * WW + UB]
                        if mm_dtype == "float32r":
                            lhsT = lhsT.bitcast(mybir.dt.float32r)
                            rhs = rhs.bitcast(mybir.dt.float32r)
                        nc.tensor.matmul(pst[:, k, 0:NMM], lhsT, rhs,
                                         start=True, stop=True)
                    # evac pair -> staging (rr, wb_local, u), (k, rr, u)->(rr, k, u)
                    src = pst[:, :, 0:NMM].rearrange(
                        "p a (r u) -> p r a u", r=R).copy()
                    dst = stv[:, :, wp * 2:wp * 2 + 2, :]
                    if evac_pattern[ev % len(evac_pattern)] == "v":
                        nc.vector.tensor_copy(dst, src)
                    else:
                        nc.scalar.copy(dst, src)
                    ev += 1
                # band window DMAs: one per rr
                for rr in range(R):
                    src = bass.AP(stt[:].tensor,
                                  (4 * rr) * ROWSZ + rr * (CHUNK * UB),
                                  [[ROWSZ, WIN], [1, CHUNK * UB]])
                    dsto = ((g * NCH + chn) * R + rr) * WIN * (CHUNK * UB)
                    dst = bass.AP(outd.ap().tensor, dsto,
                                  [[CHUNK * UB, WIN], [1, CHUNK * UB]])
                    nc.sync.dma_start(dst, src)

    nc.compile()
    return nc


def _get_nc():
    key = (MM_DTYPE, EVAC_PATTERN)
    if key not in _CACHE:
        _CACHE[key] = _build(*key)
    return _CACHE[key]


def _prep_inputs(x1, x2):
    import ml_dtypes
    np_dt = ml_dtypes.bfloat16 if MM_DTYPE == "bfloat16" else np.float32
    in_maps = []
    for b in range(B):
        x1p = np.zeros((128, HP, NWB, WW), np_dt)
        x1p.reshape(128, HP, W)[:, 8:8 + H, :] = np.asarray(x1[b], np.float32)
        win = np.stack([x1p[:, R * g:R * g + 32] for g in range(NG)], axis=1)
        x1b = win.transpose(0, 1, 3, 2, 4).reshape(128, NG * X1B)
        x2p = np.zeros((128, X2R, X2C), np_dt)
        x2p[:, 4:4 + H, 4:4 + W] = np.asarray(x2[b], np.float32)
        in_maps.append({"x1p": np.ascontiguousarray(x1b),
                        "x2p": x2p.reshape(128, X2R * X2C)})
    return in_maps


def _decode(band, out81):
    """band: per-core [OUTSZ] f32 -> out81 [81, H, W] (scaled later)."""
    arr = band.reshape(NG, NCH, R, 9, 4, CHUNK, UB)  # (g,c,rr,t,ww,wb,u)
    for ww in range(WW):
        sub = arr[:, :, :, :, ww, :, ww:ww + 9]       # (g,c,rr,t,wb,dj)
        tmat = sub.transpose(3, 5, 0, 2, 1, 4).reshape(9, 9, NG * R, NCH * CHUNK)
        for t in range(9):
            di_idx = 8 - t                            # di = 4 - t
            r2lo = di_idx                             # r2 = h + di + 4
            out81[di_idx * 9:di_idx * 9 + 9, :, ww::WW] = \
                tmat[t, :, r2lo:r2lo + H, :]
    return out81


def kernel(x1, x2):
    from concourse.bass_utils import run_bass_kernel_spmd

    x1 = np.asarray(x1, np.float32)
    x2 = np.asarray(x2, np.float32)
    nc = _get_nc()
    in_maps = _prep_inputs(x1, x2)
    res = run_bass_kernel_spmd(nc, in_maps, core_ids=list(range(8)))

    inv_sqrt_c = np.float32(1.0 / math.sqrt(C))
    out = np.empty((B, NDISP - 1, H, W), np.float32)
    out81 = np.empty((NDISP, H, W), np.float32)
    for b in range(B):
        _decode(res.results[b]["band"], out81)
        out[b] = np.delete(out81, 40, axis=0) * inv_sqrt_c
    return out
